# revision 19
# baseline (speedup 1.0000x reference)
"""Trainium2 Bass kernel: NonLocalBlock (dense spatial self-attention).

Computes, for each batch b (one NeuronCore per batch):
    xf = x[b].reshape(C, N)                       # C=144, N=4096
    q  = wq @ xf + bq                             # [16, N]
    k  = wk @ xf + bk                             # [16, N]
    v  = wv @ xf + bv                             # [C, N]
    E[n, m]   = sum_h q[h, n] k[h, m]
    attn      = softmax(E, axis=m)
    out[d, n] = gamma * sum_m v[d, m] attn[n, m] + x[d, n]

Design (per core):
  - q/k computed 4x partition-replicated (partitions 32g..32g+16 hold a
    copy + a const row) so the energy matmul uses 4-way PE row tiling
    with K=17; the 17th row injects a constant C0' so the PSUM holds
    E' = E + C0' >= 0 over the data distribution.
  - exp() is SPLIT between the Scalar engine (ACT: exp -> fp8-e5m2
    directly, bias ln K) and the Vector engine (DVE: Schraudolph
    bitcast-exp: int8 = round(5.7708 * max(E', 0)); those bits ARE the
    e5m2 encoding of K*exp(E')).  Both paths produce the same scale.
  - P@V runs "orientation B": stationary = vT tiles (cheap LDWEIGHTS),
    moving = pT, with fp8 DoubleRow (K=256: two m-blocks per matmul,
    measured 2.0x bf16 FLOPs).  Chain A: channels 0:128; chain B:
    channels 128:144 + a ones-column (-> softmax denominator row).
    Output lands in [d, n] layout - no transpose needed.
  - Normalization: denominator row [1,512] -> fp16, fast-inverse via
    bits trick (magic - bits, with gamma folded into the magic), GPSIMD
    partition_broadcast to [128,512], then one DVE STT (po * recipb)
    and one GPSIMD STT (+x) per chain.
"""

import numpy as np

B = 8
C = 144
HID = 16
N = 4096  # 64*64
NCORES = 8
P = 128
CHUNK = 512
NCHUNKS = N // CHUNK  # 8
MBLKS = N // P        # 32

C0P = 2.5                      # E' = E + C0P (const q/k row product)
A_SCHR = 4.0 / np.log(2.0)     # 5.7708 (DVE schraudolph slope)
LNK = -10.3602                 # ACT exp bias so both exp paths match
MAGIC16 = 0x7798               # fp16 fast-inverse magic
VPAD = 160                     # padded vT inner dim (stride % 16 == 0)

# exp strips: (size, psum tag, engine). 16 strips x 2 m-blocks; tags
# rotate over three 2-bank slots so three strips are in flight; engines
# alternate so ACT and DVE both stream continuously.
STRIPS = [(2, ["ea", "eb", "ec"][s % 3], "AD"[s % 2]) for s in range(16)]
NSTRIPS = len(STRIPS)
MB = [0]
for G, _, _ in STRIPS[:-1]:
    MB.append(MB[-1] + G)
assert MB[-1] + STRIPS[-1][0] == MBLKS

_CACHE = {}


def _build_nc():
    from contextlib import ExitStack

    import concourse.bass as bass
    import concourse.mybir as mybir
    import concourse.tile as tile
    from concourse import bacc, library_config
    from concourse.bass import ts

    f32 = mybir.dt.float32
    f16 = mybir.dt.float16
    f8e5 = mybir.dt.float8e5
    f8e4 = mybir.dt.float8e4
    i8 = mybir.dt.int8
    i16 = mybir.dt.int16
    AF = mybir.ActivationFunctionType
    OP = mybir.AluOpType
    DR = mybir.MatmulPerfMode.DoubleRow

    nc = bacc.Bacc("TRN2", target_bir_lowering=False, debug=False)

    x = nc.dram_tensor("x", [C, N], f32, kind="ExternalInput").ap()
    wq4a_in = nc.dram_tensor("wq4a", [P, P], f16, kind="ExternalInput").ap()
    wq4b_in = nc.dram_tensor("wq4b", [18, P], f16, kind="ExternalInput").ap()
    wk4a_in = nc.dram_tensor("wk4a", [P, P], f16, kind="ExternalInput").ap()
    wk4b_in = nc.dram_tensor("wk4b", [18, P], f16, kind="ExternalInput").ap()
    wvfa_in = nc.dram_tensor("wvfa", [P, 145], f16, kind="ExternalInput").ap()
    wvfb_in = nc.dram_tensor("wvfb", [18, 145], f16, kind="ExternalInput").ap()
    magic_in = nc.dram_tensor("magic", [1, 1], f32, kind="ExternalInput").ap()
    out = nc.dram_tensor("out", [C, N], f16, kind="ExternalOutput").ap()

    with tile.TileContext(nc) as tc, ExitStack() as ctx:
        singles = ctx.enter_context(tc.tile_pool(name="singles", bufs=1))
        work = ctx.enter_context(tc.tile_pool(name="work", bufs=2))
        psum = ctx.enter_context(tc.tile_pool(name="psum", bufs=1, space="PSUM"))

        # ------------- persistent SBUF tensors -------------
        xa = singles.tile([P, N], f32)        # x channels 0..127
        xbe = singles.tile([18, N], f32)      # row0=0, rows1:17 = x chans 128..143, row17 = ones
        xa16 = singles.tile([P, N], f16)
        xbe16 = singles.tile([18, N], f16)
        q4 = singles.tile([P, N], f16)        # q replicated; row 32g+16 = 1.0
        k4 = singles.tile([P, N], f16)        # k replicated; row 32g+16 = C0P
        vT8 = singles.tile([P, MBLKS, VPAD], f8e4)  # [m%128, m//128, d]
        gamma_sb = singles.tile([P, 1], f32)
        junk16 = singles.tile([P, 256], f16)
        nc.vector.memset(junk16, 0.0)
        shift_sb = singles.tile([P, 1], f32)  # ACT exp bias
        nc.vector.memset(shift_sb, LNK)
        nc.vector.memset(vT8[:, :, 145:VPAD], 0.0)
        nc.vector.memset(xbe[0:1, :], 0.0)

        # ------------- PE warm-up -------------
        def emit_warm(n):
            pwarm = psum.tile([P, 512], f32, tag="poA", name="pwarm")
            for _ in range(n):
                nc.tensor.matmul(pwarm[0:1, 0:256], junk16[:, 0:1], junk16,
                                 start=True, stop=True)

        emit_warm(48)

        # ------------- weight DMAs (host-prepped layouts) -------------
        wq4a = singles.tile([P, P], f16)      # [c 0..127, 32g+r] = wq[r, c]
        wq4b = singles.tile([18, P], f16)     # row0=0, rows1:17: c 128..143, row17: bias|const
        wk4a = singles.tile([P, P], f16)
        wk4b = singles.tile([18, P], f16)
        wvfa = singles.tile([P, 145], f16)    # [c 0..127, d-col] = wv[d, c]
        wvfb = singles.tile([18, 145], f16)   # row0=0, rows1:17: c 128..143, row17: bv|1.0
        magic_sb = singles.tile([1, 1], f32)
        nc.sync.dma_start(wq4a, wq4a_in)
        nc.sync.dma_start(wq4b, wq4b_in)
        nc.sync.dma_start(wk4a, wk4a_in)
        nc.sync.dma_start(wk4b, wk4b_in)
        nc.sync.dma_start(wvfa, wvfa_in)
        nc.sync.dma_start(wvfb, wvfb_in)
        nc.sync.dma_start(magic_sb, magic_in)
        ones32 = singles.tile([P, 32], f32)
        nc.vector.memset(ones32, 1.0)
        nc.sync.dma_start(
            xbe[17:18, :].rearrange("p (a b) -> p a b", a=P), ones32[:, None, :]
        )
        nc.gpsimd.load_library(library_config.attn)

        # ------------- x loads (parallel DMA queues) -------------
        dma_engines = [nc.sync, nc.scalar]
        for cc in range(NCHUNKS):
            dma_engines[cc % 2].dma_start(xa[:, ts(cc, CHUNK)], x[0:P, ts(cc, CHUNK)])
            if cc % 4 == 0:
                dma_engines[(cc // 4) % 2].dma_start(
                    xbe[1:17, ts(cc // 4, 4 * CHUNK)], x[P:C, ts(cc // 4, 4 * CHUNK)]
                )

        # ------------- helpers -------------
        def emit_cast(pc):
            nc.vector.tensor_scalar_mul(xa16[:, ts(pc, CHUNK)], xa[:, ts(pc, CHUNK)], 1.0)
            if pc % 4 == 0:
                nc.vector.tensor_scalar_mul(
                    xbe16[:, ts(pc // 4, 4 * CHUNK)], xbe[:, ts(pc // 4, 4 * CHUNK)], 1.0
                )

        def emit_proj(pc):
            pq = psum.tile([P, 512], f32, tag=["ea", "eb", "ec"][pc % 3], name=f"pq{pc}")
            nc.tensor.matmul(pq[:, 0:CHUNK], wq4a, xa16[:, ts(pc, CHUNK)], start=True, stop=False)
            nc.tensor.matmul(pq[:, 0:CHUNK], wq4b, xbe16[:, ts(pc, CHUNK)], start=False, stop=True)
            if pc % 2 == 0:
                nc.scalar.mul(q4[:, ts(pc, CHUNK)], pq[:, 0:CHUNK], 1.0)
            else:
                nc.vector.tensor_scalar_mul(q4[:, ts(pc, CHUNK)], pq[:, 0:CHUNK], 1.0)
            pk = psum.tile([P, 512], f32, tag=["eb", "ec", "ea"][pc % 3], name=f"pk{pc}")
            nc.tensor.matmul(pk[:, 0:CHUNK], wk4a, xa16[:, ts(pc, CHUNK)], start=True, stop=False)
            nc.tensor.matmul(pk[:, 0:CHUNK], wk4b, xbe16[:, ts(pc, CHUNK)], start=False, stop=True)
            if pc % 2 == 0:
                nc.vector.tensor_scalar_mul(k4[:, ts(pc, CHUNK)], pk[:, 0:CHUNK], 1.0)
            else:
                nc.scalar.mul(k4[:, ts(pc, CHUNK)], pk[:, 0:CHUNK], 1.0)

        def emit_estrip(c, s):
            G, tag, _ = STRIPS[s]
            pe = psum.tile([P, G * CHUNK], f32, tag=tag, name=f"pe{c}_{s}")
            for i in range(G):
                nc.tensor.matmul(
                    pe[:, ts(i, CHUNK)],
                    k4[32 * i: 32 * i + 17, ts(MB[s] + i, P)],
                    q4[32 * i: 32 * i + 17, ts(c, CHUNK)],
                    start=True, stop=True,
                    tile_position=(32 * i, 0),
                )
            return pe

        def emit_exp(c, s, pe, pTc):
            G, _, eng = STRIPS[s]
            dst = pTc[:, MB[s]: MB[s] + G, :]
            if eng == "A":
                nc.scalar.activation(out=dst, in_=pe, func=AF.Exp, bias=shift_sb)
            else:
                nc.vector.tensor_scalar(
                    out=dst.bitcast(i8), in0=pe,
                    scalar1=0.0, scalar2=float(A_SCHR),
                    op0=OP.max, op1=OP.mult,
                )

        # vT build, batches of 3 j-blocks through one aux psum bank
        def emit_vt_batch(j0, nj):
            pv = psum.tile([P, 3 * 145], f32, tag=["poA", "poB"][(j0 // 3) % 2], name=f"pv{j0}")
            for jj in range(nj):
                sl = pv[:, jj * 145: jj * 145 + 145]
                nc.tensor.matmul(sl, xa16[:, ts(j0 + jj, P)], wvfa, start=True, stop=False)
                nc.tensor.matmul(sl, xbe16[:, ts(j0 + jj, P)], wvfb, start=False, stop=True)
            nc.vector.tensor_scalar_mul(
                vT8[:, j0: j0 + nj, 0:145], pv[:, 0: nj * 145], 1.0
            )

        chainA = {}
        chainB = {}
        cur_cb = [None]

        def emit_metro():
            # HAM keep-warm: dummy matmul into unused partitions of the
            # most recent chain-B bank (disjoint subtile, no ordering).
            if cur_cb[0] is not None:
                nc.tensor.matmul(cur_cb[0][32:33, 0:256], junk16[:, 0:1], junk16,
                                 start=True, stop=True, tile_position=(0, 32),
                                 skip_group_check=True)

        def emit_chain_pair(c, t, pTc):
            if t == 0:
                chainA[c] = psum.tile([P, CHUNK], f32, tag="poA", name=f"poA{c}")
                chainB[c] = psum.tile([128, CHUNK], f32, tag="poB", name=f"poB{c}")
                cur_cb[0] = chainB[c]
            rhs = pTc[:, 2 * t: 2 * t + 2, :]
            nc.tensor.matmul(chainA[c], vT8[:, 2 * t: 2 * t + 2, 0:128], rhs,
                             start=(t == 0), stop=(t == 15), perf_mode=DR)
            nc.tensor.matmul(chainB[c][0:18, :], vT8[:, 2 * t: 2 * t + 2, 128:146], rhs,
                             start=(t == 0), stop=(t == 15), perf_mode=DR)


        recipbs = {}
        cps = {}

        def emit_row16(c):
            # Free the chain PSUM banks ASAP: plain high-priority copies that
            # depend only on chain completion.  Everything else (reciprocal,
            # broadcast, normalize, +x) runs later in SBUF off-path.
            cpA = work.tile([P, CHUNK], f16, tag="cpA", name=f"cpA{c}")
            nc.vector.tensor_scalar_mul(cpA, chainA[c], 1.0)
            cpB = work.tile([17, CHUNK], f16, tag="cpB", name=f"cpB{c}")
            nc.scalar.mul(cpB, chainB[c][0:17, :], 1.0)
            cps[c] = (cpA, cpB)

        def emit_rowrest(c):
            cpA, cpB = cps[c]
            rowinv = work.tile([1, CHUNK], f16, tag="rowinv", name=f"rinv{c}")
            nc.vector.tensor_scalar(
                out=rowinv.bitcast(i16), in0=cpB[0:1, :].bitcast(i16),
                scalar1=-1.0, scalar2=magic_sb,
                op0=OP.mult, op1=OP.add,
            )
            recipb = work.tile([P, CHUNK], f16, tag="recipb", name=f"recipb{c}")
            nc.gpsimd.partition_broadcast(recipb, rowinv)
            recipbs[c] = recipb

        def emit_stt(c):
            recipb = recipbs[c]
            cpA, cpB = cps[c]
            outA = work.tile([P, CHUNK], f16, tag="outA", name=f"outA{c}")
            nc.gpsimd.tensor_tensor(out=outA, in0=cpA, in1=recipb, op=OP.mult)
            outB = work.tile([17, CHUNK], f16, tag="outB", name=f"outB{c}")
            nc.gpsimd.tensor_tensor(out=outB, in0=cpB, in1=recipb[0:17, :], op=OP.mult)
            return outA, outB

        def emit_fin(c, outA, outB):
            finA = work.tile([P, CHUNK], f16, tag="finA", name=f"finA{c}")
            nc.gpsimd.tensor_tensor(out=finA, in0=outA, in1=xa16[:, ts(c, CHUNK)], op=OP.add)
            finB = work.tile([17, CHUNK], f16, tag="finB", name=f"finB{c}")
            nc.gpsimd.tensor_tensor(out=finB, in0=outB, in1=xbe16[0:17, ts(c, CHUNK)], op=OP.add)
            nc.sync.dma_start(out[0:P, ts(c, CHUNK)], finA)
            nc.sync.dma_start(out[P:C, ts(c, CHUNK)], finB[1:17, :])
            del cps[c]
            del recipbs[c]

        # ------------- chunk 0: proj + E/exp + vT + chains -------------
        pT_tiles = {}
        pT_tiles[0] = work.tile([P, MBLKS, CHUNK], f8e5, tag="pT", bufs=2, name="pT0")

        si = 0
        exp_done = 0
        vt_done = 0
        t_done = 0

        def emit_vt_avail(cast_hi):
            nonlocal vt_done
            while vt_done < MBLKS:
                nj = min(3, MBLKS - vt_done)
                if vt_done + nj > cast_hi:
                    break
                emit_vt_batch(vt_done, nj)
                vt_done += nj

        from collections import deque

        pending = deque()   # chain pairs (c, t) not yet emitted
        outq = deque()      # chunks whose out-path stages remain; (c, stage)
        sttAB = {}

        def pump_chains(c_cur, s_cur, budget=2):
            # Emit pending chain pairs whose exp strips completed >= 3 strips
            # ago, so the chain matmul never blocks the strict-FIFO PE queue.
            while pending and budget > 0:
                c2, t2 = pending[0]
                if c2 == c_cur and t2 > s_cur - 3:
                    break
                pending.popleft()
                emit_chain_pair(c2, t2, pT_tiles[c2])
                budget -= 1
                if t2 == 15:
                    emit_row16(c2)
                    outq.append((c2, 0))

        def pump_out():
            # Advance at most one out-path stage (keeps ACT/DVE/gpsimd queues
            # from head-of-line blocking on cross-engine latencies).
            if not outq:
                return
            c2, st = outq[0]
            if st == 0:
                emit_rowrest(c2)
                outq[0] = (c2, 1)
            elif st == 1:
                sttAB[c2] = emit_stt(c2)
                outq[0] = (c2, 2)
            else:
                emit_fin(c2, *sttAB[c2])
                del sttAB[c2]
                if c2 > 0:
                    del pT_tiles[c2 - 1]
                outq.popleft()

        for pc in range(NCHUNKS + 1):
            while si < NSTRIPS and (MB[si] + STRIPS[si][0] - 1) // 4 < pc:
                pe = emit_estrip(0, si)
                emit_exp(0, si, pe, pT_tiles[0])
                pending.append((0, si))
                exp_done += STRIPS[si][0]
                si += 1
                emit_vt_avail(4 * pc)
            if pc < NCHUNKS:
                emit_cast(pc)
                emit_proj(pc)
                emit_warm(2)
        assert si == NSTRIPS and exp_done == MBLKS
        emit_vt_avail(MBLKS)
        assert vt_done == MBLKS, vt_done

        # ------------- steady chunks 1..7 -------------
        for c in range(1, NCHUNKS):
            pTc = work.tile([P, MBLKS, CHUNK], f8e5, tag="pT", bufs=2, name=f"pT{c}")
            pT_tiles[c] = pTc
            pe_next = emit_estrip(c, 0)
            for s in range(NSTRIPS):
                pe = pe_next
                emit_exp(c, s, pe, pTc)
                # E-lookahead: next strip's energy matmuls go into the PE FIFO
                # before this strip's chain matmuls, so the exp engines never
                # wait on E latency.
                if s + 1 < NSTRIPS:
                    pe_next = emit_estrip(c, s + 1)
                pending.append((c, s))
                emit_metro()
                pump_out()
                pump_chains(c, s)

        # ------------- tail -------------
        c = NCHUNKS - 1
        while pending:
            pump_chains(c, NSTRIPS + 3)
            pump_out()
        while outq:
            pump_out()

    nc.finalize()
    return nc


def _get_nc():
    if "nc" not in _CACHE:
        _CACHE["nc"] = _build_nc()
    return _CACHE["nc"]


def _prep_weights(inputs):
    """Host-side packing of the tiny weight tensors into the on-chip layouts."""
    wq = np.asarray(inputs["wq"], dtype=np.float32)
    bq = np.asarray(inputs["bq"], dtype=np.float32)
    wk = np.asarray(inputs["wk"], dtype=np.float32)
    bk = np.asarray(inputs["bk"], dtype=np.float32)
    wv = np.asarray(inputs["wv"], dtype=np.float32)
    bv = np.asarray(inputs["bv"], dtype=np.float32)
    gamma = float(np.asarray(inputs["gamma"]).reshape(-1)[0])

    def pack4(w, bias, const):
        # a: [128 c, 128] with [c, 32g+r] = w[r, c];  b: [18, 128]:
        # row0 = 0, rows 1:17 = c 128..143, row 17 = bias row + const col.
        a = np.zeros((P, P), dtype=np.float16)
        bm = np.zeros((18, P), dtype=np.float16)
        for g in range(4):
            a[:, 32 * g: 32 * g + HID] = w[:, 0:P].T
            bm[1:17, 32 * g: 32 * g + HID] = w[:, P:C].T
            bm[17, 32 * g: 32 * g + HID] = bias
            bm[17, 32 * g + HID] = const
        return a, bm

    wq4a, wq4b = pack4(wq, bq, 1.0)
    wk4a, wk4b = pack4(wk, bk, C0P)

    # wvf columns: 0:128 = v-channels 0:128, col 128 = ones, 129:145 = 128:144
    wvfa = np.zeros((P, 145), dtype=np.float16)
    wvfb = np.zeros((18, 145), dtype=np.float16)
    wvfa[:, 0:P] = wv[0:P, 0:P].T
    wvfa[:, 129:145] = wv[P:C, 0:P].T
    wvfb[1:17, 0:P] = wv[0:P, P:C].T
    wvfb[1:17, 129:145] = wv[P:C, P:C].T
    wvfb[17, 0:P] = bv[0:P]
    wvfb[17, 129:145] = bv[P:C]
    wvfb[17, 128] = 1.0

    magic = np.array(
        [[MAGIC16 - 0x3C00 + int(np.float16(gamma).view(np.uint16))]],
        dtype=np.float32)

    return {"wq4a": wq4a, "wq4b": wq4b, "wk4a": wk4a, "wk4b": wk4b,
            "wvfa": wvfa, "wvfb": wvfb, "magic": magic}


def _make_in_maps(inputs):
    x = np.asarray(inputs["x"], dtype=np.float32).reshape(B, C, N)
    shared = _prep_weights(inputs)
    return [
        {"x": np.ascontiguousarray(x[b]), **shared}
        for b in range(B)
    ]


def run_spmd(inputs, trace=False, **kwargs):
    """Run on all 8 cores; returns BassKernelResults."""
    from concourse import bass_utils

    nc = _get_nc()
    in_maps = _make_in_maps(inputs)
    return bass_utils.run_bass_kernel_spmd(
        nc, in_maps, core_ids=list(range(NCORES)), trace=trace, **kwargs
    )


def kernel(**inputs) -> np.ndarray:
    res = run_spmd(inputs)
    out = np.stack([res.results[b]["out"] for b in range(B)])
    return out.reshape(B, C, 64, 64).astype(np.float32)
